# revision 4
# baseline (speedup 1.0000x reference)
"""Trainium2 Bass kernel: per-gaussian 3x3 covariance from quaternion+scale.

out_n = R_n diag((|s_n|+eps)^2) R_n^T  with R_n from normalized quaternion.

Math (fp16 on device). With raw quaternion q=(w,x,y,z), n2=|q|^2 and
half-matrix entries (column j of M = n2*R):
  m00=(w2+x2-y2-z2)/2  m10=xy+wz  m20=xz-wy     (column 0 of M/2)
  m01=xy-wz  m11=(w2+y2-x2-z2)/2  m21=yz+wx     (column 1 of M/2)
True rotation entries: r_ij = m_ij * (2/n2).  Using R R^T = I:
  out = s2^2 * I + (s0^2-s2^2) c0 c0^T + (s1^2-s2^2) c1 c1^T
with c_j = column j of R -- only TWO outer products needed, and every
intermediate is bounded (|r|<=1) so fp16 never overflows.

Engine split: ACT does all squares (one op per plane-group), DVE does fp16
tensor_tensor at 2x (packed 16-bit), Pool (gpsimd) takes ~20% of the binary
ops. Multi-plane tiles + strided access patterns fuse whole stages into
single instructions.

Layout: host packs per-partition plane-major fp16 blocks ([128, nplanes*F]),
device writes 6 unique output planes (diag first), host reassembles [N,3,3].
"""

import numpy as np

N_TOTAL = 4_000_000
N_CORES = 8
NC_RAW = N_TOTAL // N_CORES  # 500_000
P = 128
F = -(-NC_RAW // P)          # 3907 elements per partition
NC_PAD = P * F               # 500_096
PW = 832                     # tile width along free dim (5 tiles: 4x832+579)

_COMPILED = None


def _build(repeat=1):
    import contextlib
    import concourse.bacc as bacc
    import concourse.mybir as mybir
    from concourse import tile

    fp32 = mybir.dt.float32
    bf16 = mybir.dt.bfloat16
    Alu = mybir.AluOpType
    Act = mybir.ActivationFunctionType
    ISQ2 = float(1.0 / np.sqrt(2.0))

    nc = bacc.Bacc("TRN2", target_bir_lowering=False, debug=False,
                   num_devices=N_CORES)
    qt = nc.dram_tensor("qt", [P, 4 * F], bf16, kind="ExternalInput")
    st = nc.dram_tensor("st", [P, 3 * F], bf16, kind="ExternalInput")
    ot = nc.dram_tensor("ot", [P, 6 * F], bf16, kind="ExternalOutput")

    qd = qt.ap().rearrange("p (n f) -> p n f", n=4)
    sd = st.ap().rearrange("p (n f) -> p n f", n=3)
    od = ot.ap().rearrange("p (n f) -> p n f", n=6)

    def pv(t, n):
        return t[:].rearrange("p (n w) -> p n w", n=n)

    with tile.TileContext(nc) as tc:
        loop_ctx = tc.For_i(0, repeat, 1) if repeat > 1 else contextlib.nullcontext()
        with loop_ctx, tc.tile_pool(name="pool", bufs=1) as pool:
            V = nc.vector
            A = nc.scalar
            G = nc.gpsimd

            def new(tag, nplanes, dt=bf16, bufs=1):
                return pool.tile([P, nplanes * PW], dt, tag=tag, name=tag,
                                 bufs=bufs)

            off = 0
            while off < F:
                w = min(PW, F - off)
                sl = slice(off, off + w)

                QT = new("QT", 4, bufs=2)
                ST = new("ST", 3, bufs=2)
                qv = pv(QT, 4)
                sv = pv(ST, 3)
                nc.sync.dma_start(out=qv[:, :, :w], in_=qd[:, :, sl])
                nc.sync.dma_start(out=sv[:, :, :w], in_=sd[:, :, sl])

                # half-squares [w2/2|x2/2|y2/2|z2/2] and s-squares [S0|S1|S2]
                HSQ = new("HSQ", 4)
                hv = pv(HSQ, 4)
                A.activation(hv[:, :, :w], qv[:, :, :w], Act.Square, scale=ISQ2)
                SSQ = new("SSQ", 3)
                ssv = pv(SSQ, 3)
                A.activation(ssv[:, :, :w], sv[:, :, :w], Act.Square)

                # cross products [xy|xz|yz|wx|wy|wz]
                CR = new("CR", 6)
                cv = pv(CR, 6)
                V.tensor_tensor(cv[:, 0:2, :w],
                                qv[:, 1:2, :w].broadcast_to((P, 2, w)),
                                qv[:, 2:4, :w], Alu.mult)
                V.tensor_tensor(cv[:, 2:3, :w], qv[:, 2:3, :w],
                                qv[:, 3:4, :w], Alu.mult)
                V.tensor_tensor(cv[:, 3:6, :w],
                                qv[:, 0:1, :w].broadcast_to((P, 3, w)),
                                qv[:, 1:4, :w], Alu.mult)

                # quadratic combos [A|C|B|D]: A=(w2+x2)/2 C=(w2+y2)/2
                #                             B=(y2+z2)/2 D=(x2+z2)/2
                QC = new("QC", 4)
                qcv = pv(QC, 4)
                V.tensor_tensor(qcv[:, 0:2, :w],
                                hv[:, 0:1, :w].broadcast_to((P, 2, w)),
                                hv[:, 1:3, :w], Alu.add)
                V.tensor_tensor(qcv[:, 2:4, :w], hv[:, 2:0:-1, :w],
                                hv[:, 3:4, :w].broadcast_to((P, 2, w)),
                                Alu.add)

                # half-M entries ME=[m00|m10|m20|m01|m11|m21]
                ME = new("ME", 6)
                mv = pv(ME, 6)
                V.tensor_tensor(mv[:, 0:5:4, :w], qcv[:, 0:2, :w],
                                qcv[:, 2:4, :w], Alu.subtract)
                V.tensor_tensor(mv[:, 1:6:4, :w], cv[:, 0:3:2, :w],
                                cv[:, 5:2:-2, :w], Alu.add)
                V.tensor_tensor(mv[:, 2:4, :w], cv[:, 1::-1, :w],
                                cv[:, 4:6, :w], Alu.subtract)

                # n2/2 (fp32), inv = 2/n2, fp16 copy
                N2 = new("N2", 1, dt=fp32)
                n2v = pv(N2, 1)
                V.tensor_tensor(n2v[:, :, :w], qcv[:, 0:1, :w],
                                qcv[:, 2:3, :w], Alu.add)
                IV = new("IV", 1, dt=fp32)
                ivv = pv(IV, 1)
                V.reciprocal_approx_fast(out=ivv[:, :, :w], in_=n2v[:, :, :w])
                IH = new("IH", 1)
                ihv = pv(IH, 1)
                A.activation(ihv[:, :, :w], ivv[:, :, :w], Act.Copy)

                # true rotation entries RN = ME * inv
                RN = new("RN", 6)
                rv = pv(RN, 6)
                V.tensor_tensor(rv[:, :, :w], mv[:, :, :w],
                                ihv[:, 0:1, :w].broadcast_to((P, 6, w)),
                                Alu.mult)

                # P = per-column Gram pieces:
                # [r00^2|r10^2|r20^2 | r00r10|r00r20|r10r20 |
                #  r01^2|r11^2|r21^2 | r01r11|r01r21|r11r21]
                P12 = new("P12", 12, bufs=2)
                pvv = pv(P12, 12)
                p4 = P12[:].rearrange("p (a b w) -> p a b w", a=2, b=6)
                r4 = RN[:].rearrange("p (a b w) -> p a b w", a=2, b=3)
                A.activation(p4[:, :, 0:3, :w], r4[:, :, :, :w], Act.Square)
                V.tensor_tensor(pvv[:, 3:5, :w],
                                rv[:, 0:1, :w].broadcast_to((P, 2, w)),
                                rv[:, 1:3, :w], Alu.mult)
                V.tensor_tensor(pvv[:, 5:6, :w], rv[:, 1:2, :w],
                                rv[:, 2:3, :w], Alu.mult)
                V.tensor_tensor(pvv[:, 9:11, :w],
                                rv[:, 3:4, :w].broadcast_to((P, 2, w)),
                                rv[:, 4:6, :w], Alu.mult)
                V.tensor_tensor(pvv[:, 11:12, :w], rv[:, 4:5, :w],
                                rv[:, 5:6, :w], Alu.mult)

                # d_j = s_j^2 - s2^2   (Pool engine)
                DD = new("DD", 2, bufs=2)
                ddv = pv(DD, 2)
                G.tensor_tensor(ddv[:, :, :w], ssv[:, 0:2, :w],
                                ssv[:, 2:3, :w].broadcast_to((P, 2, w)),
                                Alu.subtract)

                # T_j = d_j * P_j ; T2 on Pool
                T1 = new("T1", 6)
                t1v = pv(T1, 6)
                V.tensor_tensor(t1v[:, :, :w],
                                ddv[:, 0:1, :w].broadcast_to((P, 6, w)),
                                pvv[:, 0:6, :w], Alu.mult)
                T2 = new("T2", 6, bufs=2)
                t2v = pv(T2, 6)
                G.tensor_tensor(t2v[:, :, :w],
                                ddv[:, 1:2, :w].broadcast_to((P, 6, w)),
                                pvv[:, 6:12, :w], Alu.mult)

                # diag partial sum (Pool), then final output planes
                D3 = new("D3", 3, bufs=2)
                d3v = pv(D3, 3)
                G.tensor_tensor(d3v[:, :, :w], t1v[:, 0:3, :w],
                                t2v[:, 0:3, :w], Alu.add)

                OUT = new("OUT", 6, bufs=2)
                ov = pv(OUT, 6)
                V.tensor_tensor(ov[:, 0:3, :w], d3v[:, :, :w],
                                ssv[:, 2:3, :w].broadcast_to((P, 3, w)),
                                Alu.add)
                V.tensor_tensor(ov[:, 3:6, :w], t1v[:, 3:6, :w],
                                t2v[:, 3:6, :w], Alu.add)

                nc.sync.dma_start(out=od[:, :, sl], in_=ov[:, :, :w])

                off += w

    nc.compile()
    return nc


def _get_compiled():
    global _COMPILED
    if _COMPILED is None:
        _COMPILED = _build()
    return _COMPILED


def make_in_maps(quaternion, scale):
    """Per-core input dicts: bf16, per-partition plane-major blocks."""
    import ml_dtypes
    bf = ml_dtypes.bfloat16
    q = np.asarray(quaternion)
    s = np.asarray(scale)
    in_maps = []
    for c in range(N_CORES):
        lo, hi = c * NC_RAW, (c + 1) * NC_RAW
        qp = np.zeros((4, NC_PAD), bf)
        qp[:, :NC_RAW] = q[lo:hi].T.astype(bf)
        qp[0, NC_RAW:] = 1.0  # pad with identity quaternion
        sp = np.ones((3, NC_PAD), bf)
        sp[:, :NC_RAW] = s[lo:hi].T.astype(bf)
        qtb = np.ascontiguousarray(
            qp.reshape(4, P, F).transpose(1, 0, 2)).reshape(P, 4 * F)
        stb = np.ascontiguousarray(
            sp.reshape(3, P, F).transpose(1, 0, 2)).reshape(P, 3 * F)
        in_maps.append({"qt": qtb, "st": stb})
    return in_maps


# device plane order: [c00|c11|c22|c01|c02|c12]; row-major (i,j) gather
_GATHER = [0, 3, 4, 3, 1, 5, 4, 5, 2]


def kernel(quaternion, scale):
    from concourse.bass_utils import run_bass_kernel_spmd

    q = np.asarray(quaternion, dtype=np.float32)
    s = np.asarray(scale, dtype=np.float32)
    assert q.shape == (N_TOTAL, 4) and s.shape == (N_TOTAL, 3)

    in_maps = make_in_maps(q, s)
    nc = _get_compiled()
    res = run_bass_kernel_spmd(nc, in_maps, core_ids=list(range(N_CORES)))

    out = np.empty((N_TOTAL, 3, 3), np.float32)
    for c in range(N_CORES):
        ob = res.results[c]["ot"].reshape(P, 6, F)
        o6 = ob.transpose(1, 0, 2).reshape(6, NC_PAD)
        lo = c * NC_RAW
        out[lo:lo + NC_RAW] = (
            o6[_GATHER, :NC_RAW].T.astype(np.float32).reshape(NC_RAW, 3, 3))
    return out
